# revision 46
# baseline (speedup 1.0000x reference)
"""Causal self-attention (B=4, S=2048, D=1024, H=16, hd=64) on 8 TRN2 cores.

Sharding: core c handles batch b = c//2 and head-half g = c%2 (8 heads, 512
of the 1024 qkv dims).  Host sums the two partial output projections per
batch and adds the bias.

Device kernel (per core), bf16 attention path (fp8 q/k/p/v fails the 2e-2
tolerance: attention at these magnitudes is near-uniform, so per-element
quant noise does NOT average down relative to y):
  P1: QKV projections as fp8-e4m3 DoubleRow matmuls in THREE passes
      (x_hi*W_hi + 16x_lo*W/16-ish + x/4*4W_res) which recovers bf16-level
      accuracy at half the cycles/row and double contraction depth.
      Weights are pre-scaled x32 on the host (e4m3 subnormals start at
      2^-6; the 0.02-scale weights would quantize terribly); the 32^2 is
      folded into the exp scale and /32 into Wp.
  P2: flash attention per (q-tile j, head pair a): transposed scores
      [k, q] in bf16, exp on ScalarE -> bf16 pt (diagonal chunks compute
      only the causally-valid columns), causal mask via one gpsimd
      affine_select per diagonal chunk (128-wide strip only).
      AV in the [q, d] orientation (lhsT = pt chunk columns, M=128 full):
      out [128 q, 65] per (head, q-sub) accumulated over k-chunks; the
      ones column of v gives the denominator per PARTITION, so
      normalization is reciprocal + per-partition tensor_scalar (no
      broadcast matmul).  y then goes back to [d, q] via PE transposes
      (128 cyc/tile) for the output projection.
  P3: out^T = Wp(bf16) @ yT(bf16) per q-tile, interleaved into the next
      q-tile's attention; f32 output DMA per [128, 512] tile.
"""

import numpy as np

B, S, D, H, HD = 4, 2048, 1024, 16, 64
N_CORES = 8
LH = H // 2          # local heads per core (8)
P = 128
SC = 32.0            # residual-pass scale (host)


def _ensure_concourse():
    try:
        import concourse  # noqa: F401
    except ImportError:
        import sys
        for p in ("/opt/trn_rl_repo", "/root/.axon_site/_ro/trn_rl_repo"):
            if p not in sys.path:
                sys.path.append(p)
        import concourse  # noqa: F401


def build_nc(S_=S, D_=D, LH_=LH, num_devices=N_CORES):
    """Per-core Bass program.  Requires S_%512==0, D_%128==0, LH_%2==0."""
    _ensure_concourse()
    import concourse.tile as tile
    from concourse import bacc, mybir

    f32 = mybir.dt.float32
    bf16 = mybir.dt.bfloat16
    f8 = mybir.dt.float8e4
    EXP = mybir.ActivationFunctionType.Exp
    MULT = mybir.AluOpType.mult
    IS_GE = mybir.AluOpType.is_ge
    IS_EQ = mybir.AluOpType.is_equal

    LHD = LH_ * HD            # local head dims (512)
    NPAIR = LH_ // 2          # head pairs (4)
    DCH = D_ // P             # d contraction chunks (8)
    CH = LHD // P             # P3 contraction chunks (4) == NPAIR
    NQT = S_ // 512           # q tiles (4)
    NKC = S_ // P             # k chunks (16)
    DCH2 = D_ // 256          # DoubleRow d-contraction chunks (4)
    QT = 512
    ESC = 0.125               # hd^-0.5; every fp8 pass is arranged so its
                              # product lands at NATURAL scale (x/8 (x) 8W
                              # etc), so no rescale is needed anywhere
    DRM = mybir.MatmulPerfMode.DoubleRow

    nc = bacc.Bacc("TRN2", target_bir_lowering=False, debug=False,
                   enable_asserts=True, num_devices=num_devices)

    NQT_ = S_ // 512
    # Host-side layouts are chosen so every DMA slice is CONTIGUOUS per
    # partition (one descriptor per partition): the serial DMA device is
    # descriptor-bound for row-strided patterns, so a column slice of a
    # [.., m]-major tensor costs as much as the full tensor.
    # x: [p, qtile, ko, i, 512]; wq/wk: [p, pair-group, ko, i, 256];
    # wv: [p, ko, i, m]
    x_d = [nc.dram_tensor(f"x{t}", [P, NQT_, DCH2, 2, 512], f8,
                          kind="ExternalInput").ap() for t in "abc"]
    w_d = {}
    for w in "qk":
        w_d[w] = [nc.dram_tensor(f"w{w}{t}", [P, 2, DCH2, 2, LHD // 2], f8,
                                 kind="ExternalInput").ap() for t in "abc"]
    w_d["v"] = [nc.dram_tensor(f"wv{t}", [P, DCH2, 2, LHD], f8,
                               kind="ExternalInput").ap() for t in "abc"]
    wpT = nc.dram_tensor("wpT", [LHD, D_], bf16, kind="ExternalInput").ap()
    outT = nc.dram_tensor("outT", [D_, S_], f32, kind="ExternalOutput").ap()

    x_r = x_d
    w_r = w_d
    wp_r = wpT.rearrange("(c p) d -> p c d", p=P)

    with tile.TileContext(nc) as tc:
        with tc.tile_pool(name="persist", bufs=1) as persist, \
             tc.tile_pool(name="ynp", bufs=6) as ynp, \
             tc.tile_pool(name="nrm", bufs=6) as nrm, \
             tc.tile_pool(name="ptp", bufs=16) as ptp, \
             tc.tile_pool(name="otp", bufs=4) as otp, \
             tc.tile_pool(name="ps_big", bufs=3, space="PSUM") as ps_big, \
             tc.tile_pool(name="ps_av", bufs=1, space="PSUM") as ps_av:
            # PSUM budget is 8 banks = 16KB/partition: a 3-deep 4KB ring
            # (scores pipeline + all filler tiles) and one 4KB psQ slot.
            def big_tile(name):
                return ps_big.tile([P, 2, QT], f32, tag="ss", name=name)

            # persistent SBUF
            # q and k together: [p, (q|k), pair, s] so P1 needs one copy
            qk_sb = persist.tile([P, 2, NPAIR, S_], bf16, tag="qk_sb")
            # v: [s%128, kchunk, head, 64 dims + ones column]
            v_sb = persist.tile([P, NKC, LH_, HD + 1], bf16, tag="v")
            yT = persist.tile([P, CH, S_], bf16, tag="yT")
            xt = [persist.tile([P, DCH2, 2, S_], f8, tag=f"xt{t}",
                               name=f"xt{t}") for t in "abc"]
            # wq/wk tiles are pair-group-major so a 2-pair DMA slice stays
            # contiguous per partition (sub-512B runs pay a 2x DMA penalty)
            w_t = {w: [persist.tile([P, 2, DCH2, 2, LHD // 2], f8,
                                    tag=f"w{w}{t}", name=f"w{w}{t}")
                       for t in "abc"] for w in "qk"}
            w_t["v"] = [persist.tile([P, DCH2, 2, LHD], f8, tag=f"wv{t}",
                                     name=f"wv{t}") for t in "abc"]
            wp_t = persist.tile([P, CH, D_], bf16, tag="wp")
            iden = persist.tile([P, P], bf16, tag="iden")
            ones = persist.tile([P, max(P, NKC * LH_)], bf16, tag="ones")

            nc.vector.memset(ones[:], 1.0)
            with nc.allow_low_precision(reason="exact 0/1 in bf16"):
                # transpose identity
                nc.gpsimd.affine_select(
                    out=iden[:], in_=ones[:, 0:P], pattern=[[1, P]],
                    compare_op=IS_EQ, fill=0.0, base=0, channel_multiplier=-1)
                # ones column of v (softmax denominator trick)
                nc.vector.tensor_copy(
                    v_sb[:, :, :, HD].rearrange("p a h -> p (a h)"),
                    ones[:, 0:NKC * LH_])

            # ---- input DMAs + arrival model -------------------------------
            # The cost model serializes all DMA transfers on one device
            # (~0.36 ns per byte per partition), so emission order IS the
            # service order.  dma_done[key] records the estimated completion
            # of the last DMA a compute piece needs; the drain machinery
            # only volunteers a filler once the estimated engine clock has
            # caught up to its inputs.
            dma_clock = [2000.0]   # head latency before the first transfer
            dma_done = {}

            def in_dma(dst, src, bpp, key=None):
                nc.sync.dma_start(dst, src)
                # transfers are serialized at the 360 B/ns aggregate DMA
                # bandwidth (descriptor-gen pipelines ahead); with the
                # contiguous layouts this is 0.356 ns per byte per partition
                dma_clock[0] += max(625.0, bpp * 0.356)
                if key is not None:
                    dma_done[key] = dma_clock[0]

            def x_dma(t, n, key=None):
                in_dma(xt[t][:, :, :, n * QT:(n + 1) * QT],
                       x_r[t][:, n], 4096, key)

            def wqk_dma(a2, key=None):
                # one 2-pair (256-col) slice per pass tensor (6 DMAs)
                for w in "qk":
                    for t in range(3):
                        in_dma(w_t[w][t][:, a2], w_r[w][t][:, a2], 2048, key)

            # first scores path, interleaved by first use: x pass-tensor t
            # arrives just before the pass-t weight slices so the pair-0
            # q/k projection passes can start while later passes stream in
            for t in range(3):
                x_dma(t, 0, ("x", 0) if t == 2 else None)
                for w in "qk":
                    in_dma(w_t[w][t][:, 0], w_r[w][t][:, 0], 2048,
                           ("wqk", 0) if (t, w) == (2, "k") else None)
            dma_done[("wqk", 1)] = dma_done[("wqk", 0)]
            x_dma(0, 1)
            x_dma(1, 1)
            x_dma(2, 1, ("x", 1))
            for t in range(3):
                in_dma(w_t["v"][t][:], w_r["v"][t], 4096,
                       ("wv",) if t == 2 else None)
            wqk_dma(1, ("wqk", 2))
            dma_done[("wqk", 3)] = dma_done[("wqk", 2)]
            for n in range(2, NQT):
                x_dma(0, n)
                x_dma(1, n)
                x_dma(2, n, ("x", n))
            in_dma(wp_t[:], wp_r, 8192, ("wp",))

            # estimated engine clocks: an emission-time mini-simulation of
            # the PE and ACT timelines.  Each emitted piece advances pe_ns
            # by its duration, first clamping to its input-DMA arrival (the
            # in-order PE stalls there for real); exp chains off the score
            # emission; AVs chain off their exp.  est_now() is then a good
            # absolute-time proxy for readiness decisions.
            pe_ns = [0.0]
            act_ns = [0.0]

            def bump(r):
                if r > pe_ns[0]:
                    pe_ns[0] = r

            def est_now():
                return max(pe_ns[0], act_ns[0]) + 800.0

            # ---------- emission pieces ----------
            v_done = [0] * NQT   # emitted v chunks per tile (gates AV flush)

            def p1_v(s):
                """V projection for one 128-row s-chunk (3-pass fp8 DR)."""
                psv = big_tile(f"psv_{s}")[:, 0, 0:LHD]
                last = 3 * DCH2 - 1
                for i, (xti, wvi) in enumerate(zip(xt, w_t["v"])):
                    for k in range(DCH2):
                        nc.tensor.matmul(psv[:],
                                         xti[:, k, :, s * P:(s + 1) * P],
                                         wvi[:, k, :, :],
                                         start=(i == 0 and k == 0),
                                         stop=(i * DCH2 + k == last),
                                         perf_mode=DRM)
                with nc.allow_low_precision(reason="v in bf16"):
                    nc.vector.tensor_copy(
                        v_sb[:, s, :, 0:HD],
                        psv.rearrange("p (h d) -> p h d", d=HD))
                pe_ns[0] += 1280.0
                v_done[s // 4] += 1

            def p1_qk(n, a, qi):
                """Q (qi=0) or K (qi=1) projection for q-tile n, pair a."""
                w = "qk"[qi]
                qk = big_tile(f"qk{qi}_{n}_{a}")[:, 0, :]
                last = 3 * DCH2 - 1
                g, hf = a // 2, a % 2
                for i, (xti, wti) in enumerate(zip(xt, w_t[w])):
                    for k in range(DCH2):
                        nc.tensor.matmul(
                            qk[:],
                            wti[:, g, k, :, hf * P:(hf + 1) * P],
                            xti[:, k, :, n * QT:(n + 1) * QT],
                            start=(i == 0 and k == 0),
                            stop=(i * DCH2 + k == last),
                            perf_mode=DRM)
                with nc.allow_low_precision(reason="q/k in bf16"):
                    nc.vector.tensor_copy(
                        qk_sb[:, qi, a, n * QT:(n + 1) * QT], qk[:])
                pe_ns[0] += 1280.0

            def p3_chunk(j, m):
                po = big_tile(f"po_{j}_{m}")[:, 0, :]
                for c in range(CH):
                    nc.tensor.matmul(po[:], wp_t[:, c, m * P:(m + 1) * P],
                                     yT[:, c, j * QT:(j + 1) * QT],
                                     start=(c == 0), stop=(c == CH - 1))
                ot = otp.tile([P, QT], f32, tag="ot")
                nc.vector.tensor_copy(ot[:], po[:])
                nc.sync.dma_start(outT[m * P:(m + 1) * P,
                                       j * QT:(j + 1) * QT], ot[:])
                pe_ns[0] += 853.0

            def tr_epilogue(j, a, yn):
                # y [q, d] -> yT [d, q] via PE transposes (deferred so the
                # DVE normalization has finished by the time PE gets here)
                trt = big_tile(f"tr_{j}_{a}")
                tr = trt[0:HD, :, 0:256].bitcast(bf16).rearrange(
                    "p h (q c) -> p h q c", c=P)
                for h2 in range(2):
                    for qs in range(4):
                        nc.tensor.transpose(tr[:, h2, qs, :],
                                            yn[:, h2, qs, :], iden[:])
                with nc.allow_low_precision(reason="y in bf16"):
                    nc.vector.tensor_copy(
                        yT[0:HD, a, j * QT:(j + 1) * QT].rearrange(
                            "p (q c) -> p q c", c=P),
                        tr[:, 0])
                    nc.vector.tensor_copy(
                        yT[HD:P, a, j * QT:(j + 1) * QT].rearrange(
                            "p (q c) -> p q c", c=P),
                        tr[:, 1])
                pe_ns[0] += 427.0

            # Deferred PE work (P1 pieces for the next q-tile, transposes,
            # output projection) drained one piece per k-chunk so the PE
            # stream between score matmuls stays short and the ACT engine
            # is never starved by a long un-interleaved block.  A piece is
            # only volunteered once its input DMAs are (estimated) complete:
            # the PE is in-order, so an emitted piece whose DMA is still in
            # flight blocks every later PE instruction.
            fillers = []  # (tag, ready_ns, fn)
            late = []     # (tile j, fn): output-projection pieces; a piece
                          # may only be emitted once ALL of tile j's
                          # transposes have been emitted (they write the yT
                          # columns the projection reads)
            tr_done = [0] * NQT

            # P3 pieces are deliberately held back until the LAST q-tile:
            # the front of the run is PE-bound (P1 for upcoming tiles), the
            # last tile is ACT-bound with PE slack, so the output projection
            # is the one chunk of work that can rebalance the two phases.
            in_last_tile = [False]

            def pop_late(forced=False, fallback=False):
                if late and tr_done[late[0][0]] == NPAIR and (
                        forced or ((in_last_tile[0] or fallback)
                                   and dma_done[("wp",)] <= est_now())):
                    bump(dma_done[("wp",)])
                    late.pop(0)[1]()
                    return True
                return False

            def drain_one(cur_ord=99):
                # among DMA-ready fillers pick the nearest deadline; when
                # the nearest deadline is still far away and the PE is
                # already running ahead of the exp stream, drain nothing:
                # skipping shortens a PE-bound tile and the piece lands in
                # a later tile's slack instead
                now = est_now()
                best, best_dl = None, None
                for idx, (tag, ready, dl, fn) in enumerate(fillers):
                    if ready <= now and (best is None or dl < best_dl):
                        best, best_dl = idx, dl
                if best is not None:
                    if best_dl > cur_ord + 4 and pe_ns[0] > act_ns[0]:
                        return False
                    fillers.pop(best)[3]()
                    return True
                return pop_late(fallback=True)

            def drain_tag(want):
                rest = []
                for tag, ready, dl, fn in fillers:
                    if tag == want:
                        fn()
                    else:
                        rest.append((tag, ready, dl, fn))
                fillers[:] = rest

            # software-pipeline state: closures that must run after the
            # NEXT chunk's score matmuls have been emitted (2-chunk lag so
            # the PE never waits on the exp -> affine_select chain), plus a
            # v-gate: a tile's AV pieces may only be emitted once all of the
            # tile's V projections are (the AV reads v_sb, and an AV emitted
            # first would deadlock the in-order PE queue).
            pend = []  # ("av", tile, exp_fin, fn) | ("norm", tile, 0, fn)

            def flush_av(keep=0):
                while len(pend) > keep:
                    kind, j, exp_fin, fn = pend[0]
                    if kind == "av" and v_done[j] < 4:
                        return
                    pend.pop(0)
                    bump(exp_fin + 400.0)
                    fn()

            def emit_av(psQ, pt, kc, r, a):
                for h2 in range(2):
                    hl = 2 * a + h2
                    for qs in range(max(r, 0), 4):
                        # one accumulation group per PSUM bank (h2); the
                        # start's pending-zero covers the whole bank so
                        # later qs sub-regions auto-zero.
                        nc.tensor.matmul(
                            psQ[:, h2, qs, 0:HD + 1],
                            pt[:, h2, qs * P:(qs + 1) * P],
                            v_sb[:, kc, hl, :],
                            start=(kc == 0 and qs == 0),
                            stop=(r == 3))
                pe_ns[0] += (4 - max(r, 0)) * 2 * 65 * 0.4167

            def emit_norm(j, a, psQ):
                # denominator is per-partition: one reciprocal + one
                # stride-0-broadcast multiply on DVE
                dsb = nrm.tile([P, 16], f32, tag="dsb")
                nc.vector.tensor_copy(dsb[:, 0:8],
                                      psQ[:, :, :, HD].rearrange(
                                          "p h q -> p (h q)"))
                nc.vector.reciprocal(dsb[:, 8:16], dsb[:, 0:8])
                yn = ynp.tile([P, 2, 4, HD], bf16, tag="yn")
                with nc.allow_low_precision(reason="y in bf16"):
                    nc.vector.tensor_tensor(
                        yn.rearrange("p h q c -> p (h q) c"),
                        psQ[:, :, :, 0:HD].rearrange("p h q c -> p (h q) c"),
                        dsb[:, 8:16].unsqueeze(-1).to_broadcast((P, 8, HD)),
                        MULT)
                def tr_piece(j=j, a=a, yn=yn):
                    tr_epilogue(j, a, yn)
                    tr_done[j] += 1
                fillers.append((("flash",), 0.0, 0, tr_piece))
                if j > 0:
                    late.append((j - 1, lambda m=2 * a: p3_chunk(j - 1, m)))
                    late.append((j - 1,
                                 lambda m=2 * a + 1: p3_chunk(j - 1, m)))

            def queue_qk(n, a):
                rdy = max(dma_done[("x", n)], dma_done[("wqk", a)])
                fillers.append((("p1qk", n, a), rdy, 4 * n + a,
                                lambda: (bump(rdy), p1_qk(n, a, 0))))
                fillers.append((("p1qk", n, a), rdy, 4 * n + a,
                                lambda: (bump(rdy), p1_qk(n, a, 1))))

            def queue_v(j, s):
                rdy = max(dma_done[("x", j)], dma_done[("wv",)])
                fillers.append((("p1v", j, s), rdy, 4 * j,
                                lambda: (bump(rdy), p1_v(s))))

            # P1 head: pair 0's q/k up front (first scores and the ACT exp
            # stream as early as the serial input DMA stream allows); v
            # chunks 0-3 and the other pairs' q/k drain as readiness-gated
            # fillers during tile 0.
            bump(max(dma_done[("x", 0)], dma_done[("wqk", 0)]))
            p1_qk(0, 0, 0)
            p1_qk(0, 0, 1)
            for a in range(1, NPAIR):
                queue_qk(0, a)
            for s in range(4):
                queue_v(0, s)

            for j in range(NQT):
                in_last_tile[0] = (j == NQT - 1)
                # queue next q-tile's projections behind this tile's chunks
                if j + 1 < NQT:
                    for a in range(NPAIR):
                        queue_qk(j + 1, a)
                    for s in range(4 * j + 4, 4 * j + 8):
                        queue_v(j + 1, s)
                for a in range(NPAIR):
                    drain_tag(("p1qk", j, a))
                    # psQ: [q, (head, q-sub, 64 v-dims + den + pad)]
                    psQ = ps_av.tile([P, 2, 4, P], f32, tag="psQ",
                                     name=f"psQ_{j}_{a}")
                    for kc in range(4 * j + 4):
                        r = kc - 4 * j  # diagonal index (>=0 on diagonal)
                        c0 = P * r if r >= 0 else 0
                        ss = big_tile(f"ss_{j}_{a}_{kc}")
                        for h2 in range(2):
                            o = HD * h2
                            nc.tensor.matmul(
                                ss[:, h2, c0:QT],
                                qk_sb[o:o + HD, 1, a, kc * P:(kc + 1) * P],
                                qk_sb[o:o + HD, 0, a,
                                      j * QT + c0:(j + 1) * QT],
                                start=True, stop=True)
                        pt = ptp.tile([P, 2, QT], bf16, tag="pt",
                                      name=f"pt_{j}_{a}_{kc}")
                        with nc.allow_low_precision(reason="p in bf16"):
                            nc.scalar.activation(pt[:, :, c0:QT],
                                                 ss[:, :, c0:QT],
                                                 EXP, scale=ESC)
                        if r >= 0:
                            nc.gpsimd.affine_select(
                                out=pt[:, :, c0:c0 + P],
                                in_=pt[:, :, c0:c0 + P],
                                pattern=[[0, 2], [1, P]],
                                compare_op=IS_GE, fill=0.0,
                                base=0, channel_multiplier=-1)
                        # software pipelining: the PREVIOUS chunk's AV
                        # matmuls (which wait on its exp) are emitted after
                        # this chunk's score matmuls, and a filler piece
                        # (runnable immediately) goes in front of them so
                        # the PE has work during the exp wait.  At kc==0
                        # prefer a P3 piece (no DVE work) so the previous
                        # pair's normalization is not delayed behind a
                        # projection copy on DVE (psQ is single-buffered).
                        pe_ns[0] += 2 * (QT - c0) * 0.4167
                        act_ns[0] = (max(act_ns[0], pe_ns[0])
                                     + 2 * (QT - c0) * 0.833 + 330.0)
                        if not (kc in (0, 1, 2) and pop_late()):
                            drain_one(4 * j + a)
                        # safety valve: if the AV backlog behind the v-gate
                        # nears the pt ring capacity, force the blocking v
                        # pieces in now (PE may briefly wait on their DMA)
                        if (pend and pend[0][0] == "av"
                                and v_done[pend[0][1]] < 4
                                and len(pend) >= 10):
                            jj = pend[0][1]
                            for s in range(4 * jj, 4 * jj + 4):
                                drain_tag(("p1v", jj, s))
                        flush_av(keep=1)
                        pend.append(("av", j, act_ns[0],
                                     lambda psQ=psQ, pt=pt, kc=kc, r=r, a=a:
                                     emit_av(psQ, pt, kc, r, a)))
                    # normalization + epilogue, deferred to the flush after
                    # this pair's last AV
                    pend.append(("norm", j, 0.0,
                                 lambda j=j, a=a, psQ=psQ:
                                 emit_norm(j, a, psQ)))
                    if late:
                        pop_late()
                    flush_av(keep=1)
                # end of tile: force any v leftovers in (they gate the AV
                # flush) and drain all pending AV/norm pieces
                for s in range(4 * j, 4 * j + 4):
                    drain_tag(("p1v", j, s))
                flush_av()

            for _, _, _, f in fillers:
                f()
            for _, f in late:
                f()
            for a in range(NPAIR):
                p3_chunk(NQT - 1, 2 * a)
                p3_chunk(NQT - 1, 2 * a + 1)

    nc.compile()
    return nc


class _Runner:
    """Compile once; execute the SPMD program on 8 cores via PJRT."""

    def __init__(self):
        _ensure_concourse()
        import jax
        import numpy as _np
        from jax.sharding import Mesh, PartitionSpec
        from jax.experimental.shard_map import shard_map
        from concourse import bass2jax, mybir

        self.nc = build_nc()
        bass2jax.install_neuronx_cc_hook()
        nc = self.nc

        partition_name = (nc.partition_id_tensor.name
                          if nc.partition_id_tensor else None)
        in_names, out_names, out_avals, zero_shapes = [], [], [], []
        for alloc in nc.m.functions[0].allocations:
            if not isinstance(alloc, mybir.MemoryLocationSet):
                continue
            name = alloc.memorylocations[0].name
            if alloc.kind == "ExternalInput":
                if name != partition_name:
                    in_names.append(name)
            elif alloc.kind == "ExternalOutput":
                out_names.append(name)
                shape = tuple(alloc.tensor_shape)
                dtype = mybir.dt.np(alloc.dtype)
                out_avals.append(jax.core.ShapedArray(shape, dtype))
                zero_shapes.append((shape, dtype))
        self.in_names, self.out_names = in_names, out_names
        self.out_avals, self.zero_shapes = out_avals, zero_shapes
        n_params, n_outs = len(in_names), len(out_names)

        all_in_names = in_names + out_names
        if partition_name is not None:
            all_in_names = all_in_names + [partition_name]

        def _body(*args):
            operands = list(args)
            if partition_name is not None:
                operands.append(bass2jax.partition_id_tensor())
            outs = bass2jax._bass_exec_p.bind(
                *operands,
                out_avals=tuple(out_avals),
                in_names=tuple(all_in_names),
                out_names=tuple(out_names),
                lowering_input_output_aliases=(),
                sim_require_finite=True,
                sim_require_nnan=True,
                nc=nc,
            )
            return tuple(outs)

        devices = jax.devices()[:N_CORES]
        mesh = Mesh(_np.asarray(devices), ("core",))
        donate = tuple(range(n_params, n_params + n_outs))
        self._sharded = jax.jit(
            shard_map(_body, mesh=mesh,
                      in_specs=(PartitionSpec("core"),) * (n_params + n_outs),
                      out_specs=(PartitionSpec("core"),) * n_outs,
                      check_rep=False),
            donate_argnums=donate, keep_unused=True)

    def __call__(self, in_maps):
        import numpy as _np
        concat_in = [
            _np.concatenate([in_maps[c][name] for c in range(N_CORES)], axis=0)
            for name in self.in_names
        ]
        concat_zeros = [
            _np.zeros((N_CORES * s[0], *s[1:]), dt) for s, dt in self.zero_shapes
        ]
        out_arrs = self._sharded(*concat_in, *concat_zeros)
        return [
            {name: _np.asarray(out_arrs[i]).reshape(N_CORES, *self.out_avals[i].shape)[c]
             for i, name in enumerate(self.out_names)}
            for c in range(N_CORES)
        ]


_RUNNER = None


def _get_runner():
    global _RUNNER
    if _RUNNER is None:
        _RUNNER = _Runner()
    return _RUNNER


def split3(M, scale):
    """M (f32) -> three fp8 tensors whose weighted sum reconstructs
    scale*M to ~0.05%: a + b/16 recovers x-residual, c*... see kernel doc.
    Returns (a, b, c) for the (x_hi*W_hi, x_lo*W, x*W_res) passes."""
    import ml_dtypes
    F8 = ml_dtypes.float8_e4m3
    a = (M * scale).astype(F8)
    return a


def shard_inputs(x, Wq, Wk, Wv, Wp):
    """Full inputs -> per-core input maps (host-side layout + fp8 prep).

    DRAM layouts are partition-contiguous (see build_nc): x [p,n,ko,i,c],
    wq/wk [p,g,ko,i,mc], wv [p,ko,i,m], with d = ko*256 + i*128 + p.
    """
    import ml_dtypes
    F8 = ml_dtypes.float8_e4m3
    BF16 = ml_dtypes.bfloat16
    NQT = S // 512
    DCH2 = D // 256

    def x_layout(a):  # [D, S] -> [p, n, ko, i, c]
        return np.ascontiguousarray(
            a.reshape(DCH2, 2, P, NQT, 512).transpose(2, 3, 0, 1, 4))

    def wqk_layout(a):  # [D, LHD] -> [p, g, ko, i, mc]
        return np.ascontiguousarray(
            a.reshape(DCH2, 2, P, 2, (LH * HD) // 2).transpose(2, 3, 0, 1, 4))

    def wv_layout(a):  # [D, LHD] -> [p, ko, i, m]
        return np.ascontiguousarray(
            a.reshape(DCH2, 2, P, LH * HD).transpose(2, 0, 1, 3))

    def xsplit(xt):
        # x-side pass tensors: x/8, 2*(x - 8*fp8(x/8)), x/32.  Paired with
        # the W-side scales below every pass's product is at natural scale.
        a = (xt / 8.0).astype(F8)
        b = (2.0 * (xt - 8.0 * a.astype(np.float32))).astype(F8)
        c = (xt / SC).astype(F8)
        return a, b, c

    def wsplit(wt):
        # W-side pass tensors: 8W, W/2, 32*(W - fp8(8W)/8)
        a = (8.0 * wt).astype(F8)
        b = (wt / 2.0).astype(F8)
        c = (SC * (wt - a.astype(np.float32) / 8.0)).astype(F8)
        return a, b, c

    in_maps = []
    for c in range(N_CORES):
        b, gh = c // 2, c % 2
        sl = slice(gh * LH * HD, (gh + 1) * LH * HD)
        m = {"wpT": np.ascontiguousarray(Wp[:, sl].T).astype(BF16)}
        xa, xb, xc = xsplit(np.ascontiguousarray(x[b].T))
        m["xa"], m["xb"], m["xc"] = (x_layout(t) for t in (xa, xb, xc))
        for name, W, lay in (("q", Wq, wqk_layout), ("k", Wk, wqk_layout),
                             ("v", Wv, wv_layout)):
            wa, wb, wc = wsplit(np.ascontiguousarray(W[sl, :].T))
            m[f"w{name}a"], m[f"w{name}b"], m[f"w{name}c"] = (
                lay(t) for t in (wa, wb, wc))
        in_maps.append(m)
    return in_maps


def kernel(x, Wq, Wk, Wv, Wp, bp):
    x = np.asarray(x, dtype=np.float32)
    Wq = np.asarray(Wq, dtype=np.float32)
    Wk = np.asarray(Wk, dtype=np.float32)
    Wv = np.asarray(Wv, dtype=np.float32)
    Wp = np.asarray(Wp, dtype=np.float32)
    bp = np.asarray(bp, dtype=np.float32)

    runner = _get_runner()
    outs = runner(shard_inputs(x, Wq, Wk, Wv, Wp))
    out = np.empty((B, S, D), np.float32)
    for b in range(B):
        out[b] = outs[2 * b]["outT"].T + outs[2 * b + 1]["outT"].T + bp
    return out

